# revision 59
# baseline (speedup 1.0000x reference)
"""Trainium2 Bass kernel: 3-head routed cross-entropy (moe_routing).

Math (per sample b):
    logits3[b] = hidden_state[b] @ stack(w1,w2,w3).T + stack(b1,b2,b3)   # [3, 10]
    logits[b]  = logits3[b, groups[b]]                                   # [10]
    loss       = mean_b( logsumexp(logits[b]) - logits[b, labels[b]] )

Distribution: data-parallel over 8 NeuronCores, 4096 rows each; host
finishes the scalar mean (the all-reduce of the sharding hint) in f64.

Device layout is class-major (transposed): host packs hsT chunks so the
PE matmul  psum[32, 512] += Wt_chunk[128, 32].T @ hsT_chunk[128, 512]
needs no on-device transposes.  Four 512-sample chunks are packed into
one [128, 512] PSUM tile via col-tiling (tile_position=(0, 32q)): the 4
matmul streams run concurrently on different PE column groups, and all
post-matmul work (exp, group-sum matmul, ln, masked reductions) runs on
128-partition-wide tiles, 4x fewer instructions.

Per 2048-sample superchunk:
  - 6x4 col-tiled matmuls -> logits psum [128, 512] (row j+32q = class j
    of chunk q)
  - ACT: exp(logits + bias) -> [128, 512] SBUF
  - PE:  block-diag indicator matmul [128 -> 12] = per-(chunk, group)
    sumexp
  - ACT: ln -> [12, 512]
  - DVE: scalar_tensor_tensor accumulators:
      acc_pick += sum_b (logits + bias) * onehot(sel column)
      acc_ln   += sum_b ln(sumexp) * onehot(group)
Host: loss = (sum(acc_ln) - sum(acc_pick)) / B.
"""

import sys

if "/opt/trn_rl_repo" not in sys.path:
    sys.path.insert(0, "/opt/trn_rl_repo")

import ml_dtypes
import numpy as np

import concourse.bass as bass
import concourse.mybir as mybir
import concourse.tile as tile
from concourse import bacc, bass_utils

B, H, L = 32768, 768, 10
NCORES = 8
BC = B // NCORES          # rows per core
CH = 512                  # batch columns per chunk (PSUM bank / matmul N)
NCH = BC // CH            # chunks per core
Q = 4                     # chunks packed per PSUM tile (col-tiling)
NSC = NCH // Q            # superchunks per core
M = 32                    # class rows, padded 30 -> 32
P = 128                   # SBUF partitions
KC = H // P               # contraction chunks

# hidden_state/W dtype on the wire + PE. fp8 halves HBM traffic vs bf16;
# W is pre-scaled by WSCALE on the host so its values sit in fp8's normal
# range, and the 1/WSCALE is folded into the exp's scale / host reduction.
HS_DTYPE = "fp8"          # "fp8" | "bf16" | "f32"
WSCALE = 32.0

_W_TABLE_PATCHED = False


def _dtypes():
    if HS_DTYPE == "fp8":
        dt = mybir.dt.float8e4
    elif HS_DTYPE == "bf16":
        dt = mybir.dt.bfloat16
    else:
        dt = mybir.dt.float32
    return dt, mybir.dt.np(dt)


def _patch_act_tables():
    """Prefer the LUT set holding BOTH Exp and Ln so the ACT engine loads
    one table instead of thrashing exp<->ln loads (~1.3us each)."""
    global _W_TABLE_PATCHED
    if _W_TABLE_PATCHED:
        return
    import concourse.bacc as bacc_mod

    orig = bacc_mod.get_activation_tables

    def patched(arch):
        t = orig(arch)
        if "natural_log_exp_and_others" not in t:
            return t
        # act_func_set_id is positional: keep dict order, but remove Exp/Ln
        # from every other set so the selection pass resolves both to the
        # combined set (whose LUT really holds both functions).
        exp = mybir.ActivationFunctionType.Exp
        ln = mybir.ActivationFunctionType.Ln
        return {
            k: (v if k == "natural_log_exp_and_others" else v - {exp, ln})
            for k, v in t.items()
        }

    bacc_mod.get_activation_tables = patched
    _W_TABLE_PATCHED = True


def _build_program():
    _patch_act_tables()
    dt_mm, _ = _dtypes()
    bf16 = mybir.dt.bfloat16
    f32 = mybir.dt.float32
    nc = bacc.Bacc(
        trn_type="TRN2", debug=False, enable_asserts=False, num_devices=NCORES
    )

    # hsT packed per chunk-PAIR, per-partition contiguous (6KB descriptors):
    # element (pr, p, (c*KC + kc)*CH + n) = hidden_state[(2pr+c)*CH + n, kc*P + p]
    hsT = nc.dram_tensor(
        "hsT", [NCH // 2, P, 2 * KC * CH], dt_mm, kind="ExternalInput"
    ).ap()
    wt = nc.dram_tensor("wt", [H, M], dt_mm, kind="ExternalInput").ap()
    # bias4: bias replicated over the 4 chunk slots -> [128, 1]; biasW is
    # the same scaled by WSCALE (to pair with the scaled logits in psum)
    biasd = nc.dram_tensor("bias4", [P, 1], f32, kind="ExternalInput").ap()
    biasWd = nc.dram_tensor("biasW", [P, 1], f32, kind="ExternalInput").ap()
    # ind4: block-diag [32, 3] group indicator per chunk slot -> [128, 12]
    indd = nc.dram_tensor("ind4", [P, Q * 3], bf16, kind="ExternalInput").ap()
    # sel4: [128, NSC*CH]; row j+32q, col sc*CH+n = 1{class j == sel(b)},
    # b = sc*Q*CH + q*CH + n.  oh12: [12, NSC*CH] likewise for groups.
    # Stored fp8 on the wire; SWDGE casts to bf16 during the DMA.
    fp8 = mybir.dt.float8e4
    seld = nc.dram_tensor("sel4", [P, NSC * CH], fp8, kind="ExternalInput").ap()
    oh3d = nc.dram_tensor("oh12", [Q * 3, NSC * CH], fp8, kind="ExternalInput").ap()
    # acc: cols [0, NSC) = pick sums (128 rows), cols [NSC, 2*NSC+1) = ln sums
    # (12 rows used; the last superchunk's ln is split into two half-width
    # reductions, hence the extra column) -> single output DMA
    NLC = NSC + 1
    out_acc = nc.dram_tensor("out_acc", [P, NSC + NLC], f32, kind="ExternalOutput").ap()

    wt_r = wt.rearrange("(k p) m -> p k m", p=P)     # [128, KC, M]

    add = mybir.AluOpType.add
    mult = mybir.AluOpType.mult
    Exp = mybir.ActivationFunctionType.Exp
    Ln = mybir.ActivationFunctionType.Ln

    with tile.TileContext(nc) as tc:
        with (
            tc.tile_pool(name="consts", bufs=1) as consts,
            tc.tile_pool(name="hs", bufs=NSC * Q // 2) as hpool,
            tc.tile_pool(name="work", bufs=3) as wpool,
            tc.tile_pool(name="psL", bufs=2, space="PSUM") as psL,
            tc.tile_pool(name="psG", bufs=2, space="PSUM") as psG,
        ):
            wt_sb = consts.tile([P, KC, M], dt_mm)
            nc.sync.dma_start(out=wt_sb, in_=wt_r)
            acc = consts.tile([P, NSC + NLC], f32)

            # Single Sync HWDGE ring, ordered so the critical hs chunks are
            # never starved: sc0 chunks first, tiny consts next, sc1 chunks,
            # then the (fp8, SWDGE-cast) masks which aren't needed until the
            # DVE reductions. The ACT engine issues no DMAs so its LUT state
            # stays warm after the single early table load.
            hs_sc = []
            for sc in range(NSC):
                hs_q = []
                for q0 in range(0, Q, 2):
                    hs_sb = hpool.tile([P, 2, KC, CH], dt_mm, tag="hs")
                    nc.sync.dma_start(
                        out=hs_sb,
                        in_=hsT[(sc * Q + q0) // 2].rearrange(
                            "p (c k n) -> p c k n", c=2, k=KC
                        ),
                    )
                    hs_q.append(hs_sb[:, 0])
                    hs_q.append(hs_sb[:, 1])
                hs_sc.append(hs_q)
                if sc == 0:
                    bias_sb = consts.tile([P, 1], f32)
                    nc.sync.dma_start(out=bias_sb, in_=biasd)
                    biasW_sb = consts.tile([P, 1], f32)
                    nc.sync.dma_start(out=biasW_sb, in_=biasWd)
                    ind_sb = consts.tile([P, Q * 3], bf16)
                    nc.sync.dma_start(out=ind_sb, in_=indd)
            sel_sb = consts.tile([P, NSC * CH], bf16)
            nc.gpsimd.dma_start(out=sel_sb, in_=seld)
            oh3_sb = consts.tile([Q * 3, NSC * CH], bf16)
            nc.gpsimd.dma_start(out=oh3_sb, in_=oh3d)

            psg_sc = []
            for sc in range(NSC):
                cs = slice(sc * CH, (sc + 1) * CH)
                hs_q = hs_sc[sc]
                ps = psL.tile([P, CH], f32)
                for kc in range(KC):
                    for q in range(Q):
                        nc.tensor.matmul(
                            ps[32 * q : 32 * (q + 1), :],
                            wt_sb[:, kc, :],
                            hs_q[q][:, kc, :],
                            start=(kc == 0),
                            stop=(kc == KC - 1),
                            tile_position=(0, 32 * q),
                        )

                # sum_b WSCALE*(logits + bias)[sel_b, b] for this superchunk
                junkp = wpool.tile([P, CH], f32, tag="junkp")
                nc.vector.scalar_tensor_tensor(
                    out=junkp,
                    in0=ps,
                    scalar=biasW_sb[:, :],
                    in1=sel_sb[:, cs],
                    op0=add,
                    op1=mult,
                    accum_out=acc[:, sc : sc + 1],
                )

                # exp(logits + bias), cast to bf16 for the fast indicator
                # matmul. The last superchunk is processed in two half-width
                # waves so its exp -> grp-matmul -> ln -> reduce chain
                # pipelines across ACT/PE/DVE instead of running serially.
                last = sc == NSC - 1
                halves = (
                    [slice(0, CH)]
                    if not last
                    else [slice(0, CH // 2), slice(CH // 2, CH)]
                )
                ex = wpool.tile([P, CH], bf16, tag="exp")
                psg = psG.tile([Q * 3, CH], f32)
                for h in halves:
                    nc.scalar.activation(
                        out=ex[:, h],
                        in_=ps[:, h],
                        func=Exp,
                        bias=bias_sb[:, :],
                        scale=1.0 / WSCALE,
                    )
                    nc.tensor.matmul(
                        psg[:, h], ind_sb[:, :], ex[:, h], start=True, stop=True
                    )
                psg_sc.append(psg)

            # ln phase batched after all exp phases: the ACT LUT (exp vs ln)
            # reloads cost ~1.3us each, so avoid alternating functions.
            lnc = NSC
            for sc in range(NSC):
                last = sc == NSC - 1
                halves = (
                    [slice(0, CH)]
                    if not last
                    else [slice(0, CH // 2), slice(CH // 2, CH)]
                )
                lnt = wpool.tile([Q * 3, CH], f32, tag="ln")
                junkl = wpool.tile([Q * 3, CH], f32, tag="junkl")
                for h in halves:
                    oh = slice(sc * CH + h.start, sc * CH + h.stop)
                    nc.scalar.activation(out=lnt[:, h], in_=psg_sc[sc][:, h], func=Ln)
                    nc.vector.scalar_tensor_tensor(
                        out=junkl[:, h],
                        in0=lnt[:, h],
                        scalar=0.0,
                        in1=oh3_sb[:, oh],
                        op0=add,
                        op1=mult,
                        accum_out=acc[0 : Q * 3, lnc : lnc + 1],
                    )
                    lnc += 1

            nc.sync.dma_start(out=out_acc, in_=acc)

    nc.finalize()
    return nc


def _pack_inputs(hidden_state, w1, b1, w2, b2, w3, b3, groups, labels):
    _, dt_np = _dtypes()
    bf_np = ml_dtypes.bfloat16
    hs = np.asarray(hidden_state, dtype=np.float32)
    Wpad = np.zeros((M, H), dtype=np.float32)
    Wpad[0:L] = np.asarray(w1, dtype=np.float32)
    Wpad[L : 2 * L] = np.asarray(w2, dtype=np.float32)
    Wpad[2 * L : 3 * L] = np.asarray(w3, dtype=np.float32)
    Wt = np.ascontiguousarray((Wpad.T * WSCALE).astype(dt_np))  # [H, M]

    bias1 = np.zeros(M, dtype=np.float32)
    bias1[0:L] = np.asarray(b1, dtype=np.float32)
    bias1[L : 2 * L] = np.asarray(b2, dtype=np.float32)
    bias1[2 * L : 3 * L] = np.asarray(b3, dtype=np.float32)
    bias4 = np.tile(bias1, Q)[:, None].copy()        # [128, 1]
    biasW = (bias4 * WSCALE).astype(np.float32)

    fp8_np = mybir.dt.np(mybir.dt.float8e4)
    ind1 = np.zeros((M, 3), dtype=bf_np)
    for g in range(3):
        ind1[g * L : (g + 1) * L, g] = 1.0
    ind4 = np.zeros((P, Q * 3), dtype=bf_np)         # block diag
    for q in range(Q):
        ind4[q * M : (q + 1) * M, q * 3 : (q + 1) * 3] = ind1

    groups = np.asarray(groups).astype(np.int64)
    labels = np.asarray(labels).astype(np.int64)
    col = groups * L + labels                        # [B] in [0, 30)

    hs_cast = hs.astype(dt_np)
    in_maps = []
    for c in range(NCORES):
        sl = slice(c * BC, (c + 1) * BC)
        # [BC, H] -> pair-major [NCH/2, P, 2*KC*CH]: each DMA descriptor
        # reads 6KB contiguous per partition
        hsp = (
            hs_cast[sl]
            .reshape(NCH // 2, 2, CH, KC, P)
            .transpose(0, 4, 1, 3, 2)
            .reshape(NCH // 2, P, 2 * KC * CH)
        )
        # masks in packed layout: b = sc*Q*CH + q*CH + n -> row block q, col sc*CH+n
        colc = col[sl].reshape(NSC, Q, CH)
        gc = groups[sl].reshape(NSC, Q, CH)
        n_idx = np.arange(CH)
        sel4 = np.zeros((P, NSC * CH), dtype=fp8_np)
        oh12 = np.zeros((Q * 3, NSC * CH), dtype=fp8_np)
        for sc in range(NSC):
            for q in range(Q):
                sel4[q * M + colc[sc, q], sc * CH + n_idx] = 1.0
                oh12[q * 3 + gc[sc, q], sc * CH + n_idx] = 1.0
        in_maps.append(
            {
                "hsT": np.ascontiguousarray(hsp),
                "wt": Wt,
                "bias4": bias4,
                "biasW": biasW,
                "ind4": ind4,
                "sel4": sel4,
                "oh12": oh12,
            }
        )
    return in_maps


def _run(inputs, trace=False, **kw):
    nc = _build_program()
    in_maps = _pack_inputs(**inputs)
    res = bass_utils.run_bass_kernel_spmd(
        nc, in_maps, list(range(NCORES)), trace=trace, **kw
    )
    total_ln = 0.0
    total_pick = 0.0
    for out in res.results:
        acc = out["out_acc"].astype(np.float64)
        total_pick += float(np.sum(acc[:, 0:NSC]))
        total_ln += float(np.sum(acc[0 : Q * 3, NSC:]))
    loss = (total_ln - total_pick / WSCALE) / B
    return np.float32(loss), res


def kernel(**inputs) -> np.ndarray:
    out, _ = _run(inputs, trace=False)
    return out


def benchmark(inputs, trace=True, **kw):
    """Returns (loss, BassKernelResults) with profiling enabled."""
    return _run(inputs, trace=trace, **kw)


# revision 61
# speedup vs baseline: 1.0914x; 1.0914x over previous
"""Trainium2 Bass kernel: 3-head routed cross-entropy (moe_routing).

Math (per sample b):
    logits3[b] = hidden_state[b] @ stack(w1,w2,w3).T + stack(b1,b2,b3)   # [3, 10]
    logits[b]  = logits3[b, groups[b]]                                   # [10]
    loss       = mean_b( logsumexp(logits[b]) - logits[b, labels[b]] )

Distribution: data-parallel over 8 NeuronCores, 4096 rows each; host
finishes the scalar mean (the all-reduce of the sharding hint) in f64.

Device layout is class-major (transposed): host packs hsT chunks so the
PE matmul  psum[32, 512] += Wt_chunk[128, 32].T @ hsT_chunk[128, 512]
needs no on-device transposes.  Four 512-sample chunks are packed into
one [128, 512] PSUM tile via col-tiling (tile_position=(0, 32q)): the 4
matmul streams run concurrently on different PE column groups, and all
post-matmul work (exp, group-sum matmul, ln, masked reductions) runs on
128-partition-wide tiles, 4x fewer instructions.

Per 2048-sample superchunk:
  - 6x4 col-tiled matmuls -> logits psum [128, 512] (row j+32q = class j
    of chunk q)
  - ACT: exp(logits + bias) -> [128, 512] SBUF
  - PE:  block-diag indicator matmul [128 -> 12] = per-(chunk, group)
    sumexp
  - ACT: ln -> [12, 512]
  - DVE: scalar_tensor_tensor accumulators:
      acc_pick += sum_b (logits + bias) * onehot(sel column)
      acc_ln   += sum_b ln(sumexp) * onehot(group)
Host: loss = (sum(acc_ln) - sum(acc_pick)) / B.
"""

import sys

if "/opt/trn_rl_repo" not in sys.path:
    sys.path.insert(0, "/opt/trn_rl_repo")

import ml_dtypes
import numpy as np

import concourse.bass as bass
import concourse.mybir as mybir
import concourse.tile as tile
from concourse import bacc, bass_utils

B, H, L = 32768, 768, 10
NCORES = 8
BC = B // NCORES          # rows per core
CH = 512                  # batch columns per chunk (PSUM bank / matmul N)
NCH = BC // CH            # chunks per core
Q = 4                     # chunks packed per PSUM tile (col-tiling)
NSC = NCH // Q            # superchunks per core
M = 32                    # class rows, padded 30 -> 32
P = 128                   # SBUF partitions
KC = H // P               # contraction chunks

# hidden_state/W dtype on the wire + PE. fp8 halves HBM traffic vs bf16;
# W is pre-scaled by WSCALE on the host so its values sit in fp8's normal
# range, and the 1/WSCALE is folded into the exp's scale / host reduction.
HS_DTYPE = "fp8"          # "fp8" | "bf16" | "f32"
WSCALE = 32.0

_W_TABLE_PATCHED = False


def _dtypes():
    if HS_DTYPE == "fp8":
        dt = mybir.dt.float8e4
    elif HS_DTYPE == "bf16":
        dt = mybir.dt.bfloat16
    else:
        dt = mybir.dt.float32
    return dt, mybir.dt.np(dt)


def _patch_act_tables():
    """Prefer the LUT set holding BOTH Exp and Ln so the ACT engine loads
    one table instead of thrashing exp<->ln loads (~1.3us each)."""
    global _W_TABLE_PATCHED
    if _W_TABLE_PATCHED:
        return
    import concourse.bacc as bacc_mod

    orig = bacc_mod.get_activation_tables

    def patched(arch):
        t = orig(arch)
        if "natural_log_exp_and_others" not in t:
            return t
        # act_func_set_id is positional: keep dict order, but remove Exp/Ln
        # from every other set so the selection pass resolves both to the
        # combined set (whose LUT really holds both functions).
        exp = mybir.ActivationFunctionType.Exp
        ln = mybir.ActivationFunctionType.Ln
        return {
            k: (v if k == "natural_log_exp_and_others" else v - {exp, ln})
            for k, v in t.items()
        }

    bacc_mod.get_activation_tables = patched
    _W_TABLE_PATCHED = True


def _build_program():
    _patch_act_tables()
    dt_mm, _ = _dtypes()
    bf16 = mybir.dt.bfloat16
    f32 = mybir.dt.float32
    nc = bacc.Bacc(
        trn_type="TRN2", debug=False, enable_asserts=False, num_devices=NCORES
    )

    # hsT packed per chunk-PAIR, per-partition contiguous (6KB descriptors):
    # element (pr, p, (c*KC + kc)*CH + n) = hidden_state[(2pr+c)*CH + n, kc*P + p]
    hsT = nc.dram_tensor(
        "hsT", [NCH // 2, P, 2 * KC * CH], dt_mm, kind="ExternalInput"
    ).ap()
    wt = nc.dram_tensor("wt", [H, M], dt_mm, kind="ExternalInput").ap()
    # bias4: bias replicated over the 4 chunk slots -> [128, 1]; biasW is
    # the same scaled by WSCALE (to pair with the scaled logits in psum)
    biasd = nc.dram_tensor("bias4", [P, 1], f32, kind="ExternalInput").ap()
    biasWd = nc.dram_tensor("biasW", [P, 1], f32, kind="ExternalInput").ap()
    # ind4: block-diag [32, 3] group indicator per chunk slot -> [128, 12]
    indd = nc.dram_tensor("ind4", [P, Q * 3], bf16, kind="ExternalInput").ap()
    # sel4: [128, NSC*CH]; row j+32q, col sc*CH+n = 1{class j == sel(b)},
    # b = sc*Q*CH + q*CH + n.  oh12: [12, NSC*CH] likewise for groups.
    # Stored fp8 on the wire; SWDGE casts to bf16 during the DMA.
    fp8 = mybir.dt.float8e4
    seld = nc.dram_tensor("sel4", [P, NSC * CH], fp8, kind="ExternalInput").ap()
    oh3d = nc.dram_tensor("oh12", [Q * 3, NSC * CH], fp8, kind="ExternalInput").ap()
    # acc: cols [0, NSC) = pick sums (128 rows), cols [NSC, 2*NSC+1) = ln sums
    # (12 rows used; the last superchunk's ln is split into two half-width
    # reductions, hence the extra column) -> single output DMA
    NLC = NSC + 1
    out_acc = nc.dram_tensor("out_acc", [P, NSC + NLC], f32, kind="ExternalOutput").ap()

    wt_r = wt.rearrange("(k p) m -> p k m", p=P)     # [128, KC, M]

    add = mybir.AluOpType.add
    mult = mybir.AluOpType.mult
    Exp = mybir.ActivationFunctionType.Exp
    Ln = mybir.ActivationFunctionType.Ln

    with tile.TileContext(nc) as tc:
        with (
            tc.tile_pool(name="consts", bufs=1) as consts,
            tc.tile_pool(name="hs", bufs=NSC * Q // 2) as hpool,
            tc.tile_pool(name="work", bufs=3) as wpool,
            tc.tile_pool(name="psL", bufs=2, space="PSUM") as psL,
            tc.tile_pool(name="psG", bufs=2, space="PSUM") as psG,
        ):
            wt_sb = consts.tile([P, KC, M], dt_mm)
            nc.sync.dma_start(out=wt_sb, in_=wt_r)
            acc = consts.tile([P, NSC + NLC], f32)

            # Single Sync HWDGE ring, ordered so the critical hs chunks are
            # never starved: sc0 chunks first, tiny consts next, sc1 chunks,
            # then the (fp8, SWDGE-cast) masks which aren't needed until the
            # DVE reductions. The ACT engine issues no DMAs so its LUT state
            # stays warm after the single early table load.
            hs_sc = []
            for sc in range(NSC):
                hs_q = []
                for q0 in range(0, Q, 2):
                    hs_sb = hpool.tile([P, 2, KC, CH], dt_mm, tag="hs")
                    nc.sync.dma_start(
                        out=hs_sb,
                        in_=hsT[(sc * Q + q0) // 2].rearrange(
                            "p (c k n) -> p c k n", c=2, k=KC
                        ),
                    )
                    hs_q.append(hs_sb[:, 0])
                    hs_q.append(hs_sb[:, 1])
                hs_sc.append(hs_q)
                if sc == 0:
                    bias_sb = consts.tile([P, 1], f32)
                    nc.sync.dma_start(out=bias_sb, in_=biasd)
                    biasW_sb = consts.tile([P, 1], f32)
                    nc.sync.dma_start(out=biasW_sb, in_=biasWd)
                    ind_sb = consts.tile([P, Q * 3], bf16)
                    nc.sync.dma_start(out=ind_sb, in_=indd)
            sel_sb = consts.tile([P, NSC * CH], bf16)
            nc.gpsimd.dma_start(out=sel_sb, in_=seld)
            oh3_sb = consts.tile([Q * 3, NSC * CH], bf16)
            nc.gpsimd.dma_start(out=oh3_sb, in_=oh3d)

            psg_sc = []
            for sc in range(NSC):
                cs = slice(sc * CH, (sc + 1) * CH)
                hs_q = hs_sc[sc]
                ps = psL.tile([P, CH], f32)
                for kc in range(KC):
                    for q in range(Q):
                        nc.tensor.matmul(
                            ps[32 * q : 32 * (q + 1), :],
                            wt_sb[:, kc, :],
                            hs_q[q][:, kc, :],
                            start=(kc == 0),
                            stop=(kc == KC - 1),
                            tile_position=(0, 32 * q),
                        )

                # sum_b WSCALE*(logits + bias)[sel_b, b] for this superchunk
                junkp = wpool.tile([P, CH], f32, tag="junkp")
                nc.vector.scalar_tensor_tensor(
                    out=junkp,
                    in0=ps,
                    scalar=biasW_sb[:, :],
                    in1=sel_sb[:, cs],
                    op0=add,
                    op1=mult,
                    accum_out=acc[:, sc : sc + 1],
                )

                # exp(logits + bias), cast to bf16 for the fast indicator
                # matmul. The last superchunk is processed in two half-width
                # waves so its exp -> grp-matmul -> ln -> reduce chain
                # pipelines across ACT/PE/DVE instead of running serially.
                last = sc == NSC - 1
                halves = (
                    [slice(0, CH)]
                    if not last
                    else [slice(0, CH // 2), slice(CH // 2, CH)]
                )
                ex = wpool.tile([P, CH], bf16, tag="exp")
                psg = psG.tile([Q * 3, CH], f32)
                for h in halves:
                    nc.scalar.activation(
                        out=ex[:, h],
                        in_=ps[:, h],
                        func=Exp,
                        bias=bias_sb[:, :],
                        scale=1.0 / WSCALE,
                    )
                    nc.tensor.matmul(
                        psg[:, h], ind_sb[:, :], ex[:, h], start=True, stop=True
                    )
                psg_sc.append(psg)

            # ln phase batched after all exp phases: the ACT LUT (exp vs ln)
            # reloads cost ~1.3us each, so avoid alternating functions.
            lnc = NSC
            for sc in range(NSC):
                last = sc == NSC - 1
                halves = (
                    [slice(0, CH)]
                    if not last
                    else [slice(0, CH // 2), slice(CH // 2, CH)]
                )
                lnt = wpool.tile([Q * 3, CH], f32, tag="ln")
                junkl = wpool.tile([Q * 3, CH], f32, tag="junkl")
                for h in halves:
                    oh = slice(sc * CH + h.start, sc * CH + h.stop)
                    nc.scalar.activation(out=lnt[:, h], in_=psg_sc[sc][:, h], func=Ln)
                    nc.vector.scalar_tensor_tensor(
                        out=junkl[:, h],
                        in0=lnt[:, h],
                        scalar=0.0,
                        in1=oh3_sb[:, oh],
                        op0=add,
                        op1=mult,
                        accum_out=acc[0 : Q * 3, lnc : lnc + 1],
                    )
                    lnc += 1

            nc.sync.dma_start(out=out_acc, in_=acc)

    nc.finalize()
    return nc


def _pack_inputs(hidden_state, w1, b1, w2, b2, w3, b3, groups, labels):
    _, dt_np = _dtypes()
    bf_np = ml_dtypes.bfloat16
    hs = np.asarray(hidden_state, dtype=np.float32)
    Wpad = np.zeros((M, H), dtype=np.float32)
    Wpad[0:L] = np.asarray(w1, dtype=np.float32)
    Wpad[L : 2 * L] = np.asarray(w2, dtype=np.float32)
    Wpad[2 * L : 3 * L] = np.asarray(w3, dtype=np.float32)
    Wt = np.ascontiguousarray((Wpad.T * WSCALE).astype(dt_np))  # [H, M]

    bias1 = np.zeros(M, dtype=np.float32)
    bias1[0:L] = np.asarray(b1, dtype=np.float32)
    bias1[L : 2 * L] = np.asarray(b2, dtype=np.float32)
    bias1[2 * L : 3 * L] = np.asarray(b3, dtype=np.float32)
    bias4 = np.tile(bias1, Q)[:, None].copy()        # [128, 1]
    biasW = (bias4 * WSCALE).astype(np.float32)

    fp8_np = mybir.dt.np(mybir.dt.float8e4)
    ind1 = np.zeros((M, 3), dtype=bf_np)
    for g in range(3):
        ind1[g * L : (g + 1) * L, g] = 1.0
    ind4 = np.zeros((P, Q * 3), dtype=bf_np)         # block diag
    for q in range(Q):
        ind4[q * M : (q + 1) * M, q * 3 : (q + 1) * 3] = ind1

    groups = np.asarray(groups).astype(np.int64)
    labels = np.asarray(labels).astype(np.int64)
    col = groups * L + labels                        # [B] in [0, 30)

    hs_cast = hs.astype(dt_np)
    in_maps = []
    for c in range(NCORES):
        sl = slice(c * BC, (c + 1) * BC)
        # [BC, H] -> pair-major [NCH/2, P, 2*KC*CH]: each DMA descriptor
        # reads 6KB contiguous per partition
        hsp = (
            hs_cast[sl]
            .reshape(NCH // 2, 2, CH, KC, P)
            .transpose(0, 4, 1, 3, 2)
            .reshape(NCH // 2, P, 2 * KC * CH)
        )
        # masks in packed layout: b = sc*Q*CH + q*CH + n -> row block q, col sc*CH+n
        colc = col[sl].reshape(NSC, Q, CH)
        gc = groups[sl].reshape(NSC, Q, CH)
        n_idx = np.arange(CH)
        sel4 = np.zeros((P, NSC * CH), dtype=fp8_np)
        oh12 = np.zeros((Q * 3, NSC * CH), dtype=fp8_np)
        for sc in range(NSC):
            for q in range(Q):
                sel4[q * M + colc[sc, q], sc * CH + n_idx] = 1.0
                oh12[q * 3 + gc[sc, q], sc * CH + n_idx] = 1.0
        in_maps.append(
            {
                "hsT": np.ascontiguousarray(hsp),
                "wt": Wt,
                "bias4": bias4,
                "biasW": biasW,
                "ind4": ind4,
                "sel4": sel4,
                "oh12": oh12,
            }
        )
    return in_maps


def _run(inputs, trace=False, **kw):
    nc = _build_program()
    in_maps = _pack_inputs(**inputs)
    res = bass_utils.run_bass_kernel_spmd(
        nc, in_maps, list(range(NCORES)), trace=trace, **kw
    )
    total_ln = 0.0
    total_pick = 0.0
    for out in res.results:
        acc = out["out_acc"].astype(np.float64)
        total_pick += float(np.sum(acc[:, 0:NSC]))
        total_ln += float(np.sum(acc[0 : Q * 3, NSC:]))
    loss = (total_ln - total_pick / WSCALE) / B
    return np.float32(loss), res


def kernel(**inputs) -> np.ndarray:
    out, _ = _run(inputs, trace=False)
    return out


def benchmark(inputs, trace=True, **kw):
    """Returns (loss, BassKernelResults) with profiling enabled."""
    return _run(inputs, trace=trace, **kw)
